# revision 56
# baseline (speedup 1.0000x reference)
"""NCC loss (9x9x9 box normalized cross-correlation) on 8 TRN2 NeuronCores.

Inputs: y_pred, y_true f32 (2,1,128,128,128). Output: scalar f32 loss.

Sharding: D axis (dim 2) split 4-ways per batch -> 8 slabs of 32 D-slices,
each with a 4-slice halo (host zero-pads volume edges).

Per core, fp16 on-chip with f32 PSUM, band taps = 1/9 so every pass emits
window MEANS (the tap scale cancels exactly in cc = cross^2/(Iv*Jv)):
  vols   : I, J (fp16, DMA'd in [h,d,w] layout), I*I, J*J, I*J (DVE/Pool)
  pass 1 : per-d-slice flip matmul vs 9-band bh -> H box   [W, (h', d)]
  pass 2 : per-group flip matmul vs bh          -> W box   [(l,d), (g, w)]
  pass 3 : block-band b3 [120,96]               -> D box, f32 PSUM
  ptw    : cc per voxel, accumulated via STT accum_out
Passes 2+3+pointwise run fused per 4-group block so ACT/DVE/Pool pointwise
overlaps PE matmuls. Host: sum per-core partials, loss = -sum / N.

Group scheme: 43 groups of 3 h'-slices; groups 0..41 cover h' 0..125,
group 42 covers h' 126..127 (band rows limited to 80/64).
"""

import numpy as np

import concourse.bacc as bacc
import concourse.tile as tile
from concourse import mybir
from concourse.bass_utils import run_bass_kernel_spmd

F32 = mybir.dt.float32
FP16 = mybir.dt.float16
ALU = mybir.AluOpType
ACTF = mybir.ActivationFunctionType

B, Dv, H, W = 2, 128, 128, 128
DL, PAD = 32, 4
DH = DL + 2 * PAD            # 40
NG = 43
TAP = 1.0 / 9.0
N_TOT = float(B * Dv * H * W)

_CACHE = {}


def _build():
    nc = bacc.Bacc(trn_type="TRN2", target_bir_lowering=False)

    # host uploads fp16 slabs already transposed to [h, d, w] so DMA runs
    # are (d,w)-contiguous (>=512B descriptors)
    i_dram = nc.dram_tensor("i_slab", [H, DH, W], FP16, kind="ExternalInput")
    j_dram = nc.dram_tensor("j_slab", [H, DH, W], FP16, kind="ExternalInput")
    out_dram = nc.dram_tensor("partials", [128, 1], F32, kind="ExternalOutput")

    with tile.TileContext(nc) as tc:
        with (
            tc.tile_pool(name="bands", bufs=1) as bands,
            tc.tile_pool(name="t2", bufs=1) as t2p,
            tc.tile_pool(name="accp", bufs=1) as accp,
        ):
            # ---------- band matrices (taps 1/9) ----------
            bh = bands.tile([128, 128], FP16)
            nc.gpsimd.memset(bh[:, :], TAP)
            nc.gpsimd.affine_select(bh[:, :], bh[:, :], pattern=[[-1, 128]],
                                    compare_op=ALU.is_ge, fill=0.0,
                                    base=PAD, channel_multiplier=1)
            nc.gpsimd.affine_select(bh[:, :], bh[:, :], pattern=[[1, 128]],
                                    compare_op=ALU.is_ge, fill=0.0,
                                    base=PAD, channel_multiplier=-1)
            b3 = bands.tile([128, 3, 32], FP16)
            nc.gpsimd.memset(b3[:, :, :], TAP)
            nc.gpsimd.affine_select(b3[:, :, :], b3[:, :, :],
                                    pattern=[[-40, 3], [-1, 32]],
                                    compare_op=ALU.is_ge, fill=0.0,
                                    base=0, channel_multiplier=1)
            nc.gpsimd.affine_select(b3[:, :, :], b3[:, :, :],
                                    pattern=[[40, 3], [1, 32]],
                                    compare_op=ALU.is_ge, fill=0.0,
                                    base=8, channel_multiplier=-1)
            nc.gpsimd.affine_select(b3[:, :, :], b3[:, :, :],
                                    pattern=[[0, 3], [0, 32]],
                                    compare_op=ALU.is_ge, fill=0.0,
                                    base=119, channel_multiplier=-1)
            b3f = b3.rearrange("p l j -> p (l j)")

            acc_all = accp.tile([128, 12], F32)
            nc.vector.memset(acc_all[:, :], 0.0)

            # ---------- t1 for all 5 vols lives through the fused loop ----
            cm_t1 = tc.tile_pool(name="t1", bufs=1)
            t1p = cm_t1.__enter__()
            t1s = [t1p.tile([128, 128, DH], FP16, tag=f"t1_{v}",
                            name=f"t1_{v}") for v in range(5)]

            # ---------- load fp16 vols + products ----------
            cm_vol = tc.tile_pool(name="vols", bufs=1)
            volsp = cm_vol.__enter__()

            vols = [volsp.tile([128, DH, W], FP16, tag=f"vol{v}",
                               name=f"vol{v}") for v in range(5)]
            vI, vJ, vI2, vJ2, vIJ = vols
            for q in range(4):
                s = slice(q * 10, q * 10 + 10)
                nc.sync.dma_start(out=vI[:, s, :], in_=i_dram[:, s, :])
            for q in range(4):
                s = slice(q * 10, q * 10 + 10)
                nc.sync.dma_start(out=vJ[:, s, :], in_=j_dram[:, s, :])
                nc.vector.tensor_tensor(out=vI2[:, s, :], in0=vI[:, s, :],
                                        in1=vI[:, s, :], op=ALU.mult)
                nc.gpsimd.tensor_tensor(out=vJ2[:, s, :], in0=vJ[:, s, :],
                                        in1=vJ[:, s, :], op=ALU.mult)
                nc.vector.tensor_tensor(out=vIJ[:, s, :], in0=vI[:, s, :],
                                        in1=vJ[:, s, :], op=ALU.mult)

            # ---------- pass 1, all vols ----------
            cm_ps1 = tc.tile_pool(name="ps1", bufs=4, space="PSUM")
            ps1p = cm_ps1.__enter__()
            rr = [0]

            def copy_rr(dst, src):
                # GPSIMD cannot touch PSUM: alternate ACT/DVE
                k = rr[0] % 2
                rr[0] += 1
                if k == 0:
                    nc.scalar.copy(dst, src)
                else:
                    nc.vector.tensor_copy(dst, src)

            for v in range(5):
                for db in range(5):
                    nd = 8
                    d0 = db * 8
                    ps = ps1p.tile([128, 8, 128], F32, tag="ps1")
                    for k in range(nd):
                        nc.tensor.matmul(out=ps[:, k, :],
                                         lhsT=vols[v][:, d0 + k, :],
                                         rhs=bh[:, :])
                    dd = slice(d0, d0 + nd)
                    copy_rr(t1s[v][:, :, dd].rearrange("p h d -> p d h"),
                            ps[:, 0:nd, :], mod=5, act=3)
            cm_ps1.__exit__(None, None, None)
            cm_vol.__exit__(None, None, None)

            # ---------- fused pass2 + pass3 + pointwise per 4-group block --
            cm_ps2 = tc.tile_pool(name="ps2", bufs=3, space="PSUM")
            ps2p = cm_ps2.__enter__()
            cm_ps3 = tc.tile_pool(name="ps3", bufs=5, space="PSUM")
            ps3p = cm_ps3.__enter__()
            cm_ptw = tc.tile_pool(name="ptw", bufs=4)
            ptw = cm_ptw.__enter__()

            t2 = [t2p.tile([128, NG, 128], FP16, tag=f"t2_{v}",
                           name=f"t2_{v}") for v in range(5)]

            def pass2_block(v, gs):
                """Groups gs (<=4) of vol v -> t2[v]."""
                ps = ps2p.tile([128, 4, 128], F32, tag="ps2")
                for k, g in enumerate(gs):
                    h0, hn = (3 * g, 3) if g < 42 else (126, 2)
                    lhs = t1s[v][:, h0:h0 + hn, :].rearrange("p l d -> p (l d)")
                    nc.tensor.matmul(out=ps[0:hn * DH, k, :],
                                     lhsT=lhs, rhs=bh[:, :])
                n_full = sum(1 for g in gs if g < 42)
                if n_full:
                    copy_rr(t2[v][0:120, gs[0]:gs[0] + n_full, :],
                            ps[0:120, 0:n_full, :])
                if gs[-1] == 42:
                    copy_rr(t2[v][0:80, 42, :], ps[0:80, len(gs) - 1, :])

            def p3_mm(v, g0, ng, P, F, Kk, lhs3):
                pt = ps3p.tile([96, 512], F32, tag="ps3")
                nc.tensor.matmul(
                    out=pt[0:P, 0:F],
                    lhsT=lhs3,
                    rhs=t2[v][0:Kk, g0:g0 + ng, :].rearrange(
                        "p g w -> p (g w)"))
                return pt

            def ptw_chunk(ci, g0, ng, P, F, Kk, lhs3, ps5=None):
                if ps5 is None:
                    ps5 = [p3_mm(v, g0, ng, P, F, Kk, lhs3)
                           for v in range(5)]
                sA = ps5[0][0:P, 0:F]
                sB = ps5[1][0:P, 0:F]
                sC = ps5[2][0:P, 0:F]
                sD = ps5[3][0:P, 0:F]
                sE = ps5[4][0:P, 0:F]

                def st(tag, dt=FP16):
                    return ptw.tile([96, 512], dt, tag=tag,
                                    name=tag)[0:P, 0:F]

                qA, qB, bB, bC, bD = (st("qA"), st("qB"), st("bB"),
                                      st("bC"), st("bD"))
                nc.scalar.activation(qA, sA, ACTF.Square)
                nc.scalar.copy(bC, sC)
                nc.scalar.activation(qB, sB, ACTF.Square)
                nc.scalar.copy(bD, sD)
                nc.scalar.copy(bB, sB)

                Pm, cross, num = st("Pm"), st("cross"), st("num")
                nc.vector.scalar_tensor_tensor(out=Pm, in0=sA, scalar=1.0,
                                               in1=bB, op0=ALU.bypass,
                                               op1=ALU.mult)
                nc.vector.scalar_tensor_tensor(out=cross, in0=sE, scalar=1.0,
                                               in1=Pm, op0=ALU.bypass,
                                               op1=ALU.subtract)
                nc.vector.tensor_tensor(out=num, in0=cross, in1=cross,
                                        op=ALU.mult)
                Iv, Jv, dene = st("Iv"), st("Jv"), st("dene", F32)
                nc.gpsimd.tensor_tensor(out=Iv, in0=bC, in1=qA,
                                        op=ALU.subtract)
                nc.gpsimd.tensor_tensor(out=Jv, in0=bD, in1=qB,
                                        op=ALU.subtract)
                nc.gpsimd.tensor_tensor(out=dene, in0=Iv, in1=Jv,
                                        op=ALU.mult)
                rec = st("rec", F32)
                nc.vector.reciprocal_approx_fast(out=rec, in_=dene)
                ccs = st("ccs")
                nc.vector.scalar_tensor_tensor(
                    out=ccs, in0=num, scalar=1.0, in1=rec,
                    op0=ALU.bypass, op1=ALU.mult,
                    accum_out=acc_all[0:P, ci:ci + 1])

            for ci in range(10):
                gs = list(range(ci * 4, ci * 4 + 4))
                ps5 = []
                for v in range(5):
                    pass2_block(v, gs)
                    ps5.append(p3_mm(v, ci * 4, 4, 96, 512, 120,
                                     b3f[0:120, 0:96]))
                ptw_chunk(ci, ci * 4, 4, 96, 512, 120, b3f[0:120, 0:96],
                          ps5=ps5)
            # groups 40..42 then chunks 10, 11
            for v in range(5):
                pass2_block(v, [40, 41, 42])
            ptw_chunk(10, 40, 2, 96, 256, 120, b3f[0:120, 0:96])
            ptw_chunk(11, 42, 1, 64, 128, 80, b3f[0:80, 0:64])

            cm_ptw.__exit__(None, None, None)
            cm_ps3.__exit__(None, None, None)
            cm_ps2.__exit__(None, None, None)
            cm_t1.__exit__(None, None, None)

            accs = accp.tile([128, 1], F32)
            nc.vector.tensor_reduce(out=accs[:, :], in_=acc_all[:, :],
                                    axis=mybir.AxisListType.X, op=ALU.add)
            nc.sync.dma_start(out=out_dram[:, :], in_=accs[:, :])

    nc.compile()
    return nc


def kernel(y_pred: np.ndarray, y_true: np.ndarray) -> np.ndarray:
    y_pred = np.ascontiguousarray(np.asarray(y_pred, dtype=np.float32))
    y_true = np.ascontiguousarray(np.asarray(y_true, dtype=np.float32))

    if "nc" not in _CACHE:
        _CACHE["nc"] = _build()
    nc = _CACHE["nc"]

    in_maps = []
    for core in range(8):
        b = core // 4
        d0 = (core % 4) * DL
        islab = np.zeros((DH, H, W), np.float16)
        jslab = np.zeros((DH, H, W), np.float16)
        lo, hi = d0 - PAD, d0 + DL + PAD
        slo, shi = max(lo, 0), min(hi, Dv)
        islab[slo - lo:shi - lo] = y_true[b, 0, slo:shi]
        jslab[slo - lo:shi - lo] = y_pred[b, 0, slo:shi]
        in_maps.append({
            "i_slab": np.ascontiguousarray(islab.transpose(1, 0, 2)),
            "j_slab": np.ascontiguousarray(jslab.transpose(1, 0, 2)),
        })

    res = run_bass_kernel_spmd(nc, in_maps, core_ids=list(range(8)))
    total = 0.0
    for r in res.results:
        total += float(np.asarray(r["partials"], np.float64).sum())
    return np.float32(-total / N_TOT)


if __name__ == "__main__":
    rng = np.random.default_rng(0)
    yp = rng.standard_normal((B, 1, Dv, H, W), dtype=np.float32)
    yt = rng.standard_normal((B, 1, Dv, H, W), dtype=np.float32)
    print("loss:", kernel(yp, yt))
